# revision 3
# baseline (speedup 1.0000x reference)
"""Trainium2 Bass kernel for nn_AdamLayer (moe_routing) — data-parallel sparse.

Strategy (8 NeuronCores, SPMD, zero collectives):
  - Data-parallel: core i owns tokens [i*512, (i+1)*512) and runs ALL 8 experts
    locally on just the tokens routed to each (capacity 176 of mean 128 per
    expert). Expert weights stream from HBM through double-buffered SBUF
    tiles, overlapped with the previous expert's FFN.
  - Gated sum accumulates across experts in PSUM via deferred e-outer scatter
    matmuls into 4 persistent PSUM banks (one per token tile); the fused
    Adam+LayerNorm epilogue runs per token tile as soon as its accumulation
    closes, overlapping the FFN tail and spreading the output DMAs.
  - b2 bias enters through a single K=8 matmul per token tile
    (gate^T @ b2-matrix) that also opens the PSUM accumulation, replacing
    per-expert K=1 broadcast matmuls.
  - Token compaction is matmul-based: prefix-sum slot ids via a triangular
    matmul, 0/1 selection matrix for the gather, gate-weighted transposed
    selection for the scatter; slot-id/gate broadcast along the free axis
    bounces through DRAM once per core (in bf16).

Math notes: y = x - adam = -p_new/sqrt(v_new+eps) (x cancels; sign folded into a
negated ln_w tile); b1 rides the relu as its per-partition bias.
"""

import numpy as np
import ml_dtypes

import concourse.bass as bass
import concourse.mybir as mybir
from concourse import bacc
import concourse.tile as tile
from concourse.bass_utils import run_bass_kernel_spmd

# Problem constants (hardcoded per harness contract)
B, S, D, H, E = 2, 2048, 512, 2048, 8
T = B * S                  # 4096 tokens
NCORES = 8
TC = T // NCORES           # 512 tokens per core
NTT = TC // 128            # 4 token tiles
KD = D // 128              # 4 contraction tiles over D
KH = H // 128              # 16 contraction tiles over H
CAP = 176                  # capacity per expert (mean 128, sigma ~9.8: +4.9σ)
NSL = 2                    # slot tiles per expert (128 + 48)
ECAP = E * CAP             # 1408 total slots
BIG = 65536.0              # slot id for unrouted tokens

MU, G1, G2, BETA1, BETA2 = 0.7, 1.0, 1.0, 0.9, 0.999
EPS_ADAM = 1e-8
EPS_LN = 1e-5

F32 = mybir.dt.float32
BF16 = mybir.dt.bfloat16
AX = mybir.AxisListType
ALU = mybir.AluOpType
ACTF = mybir.ActivationFunctionType


def _bcast_last(ap: bass.AP, n: int) -> bass.AP:
    """View a [..., 1] AP as [..., n] via a step-0 innermost dim."""
    return bass.AP(tensor=ap.tensor, offset=ap.offset, ap=[*ap.ap[:-1], [0, n]])


def _bcast_part(ap: bass.AP, parts: int) -> bass.AP:
    """View a [1, ...] AP as [parts, ...] via a step-0 partition dim."""
    return bass.AP(tensor=ap.tensor, offset=ap.offset, ap=[[0, parts], *ap.ap[1:]])


def build_graph() -> bass.Bass:
    nc = bacc.Bacc(None, num_devices=NCORES)

    # ---- per-core kernel I/O ----
    xT = nc.declare_dram_parameter("xT", [D, TC], F32, isOutput=False)   # x^T shard
    xb = nc.declare_dram_parameter("xb", [TC, D], BF16, isOutput=False)  # x shard bf16
    wcat = nc.declare_dram_parameter(
        "wcat", [E, 128, KD * H + KH * D], BF16, isOutput=False)
    b1c = nc.declare_dram_parameter("b1c", [128, E * KH], F32, isOutput=False)
    b2r = nc.declare_dram_parameter("b2r", [E, D], BF16, isOutput=False)
    gw = nc.declare_dram_parameter("gw", [D, E], F32, isOutput=False)
    gbr = nc.declare_dram_parameter("gbr", [1, E], F32, isOutput=False)
    lnw = nc.declare_dram_parameter("lnw", [1, D], F32, isOutput=False)
    lnb = nc.declare_dram_parameter("lnb", [1, D], F32, isOutput=False)
    tri = nc.declare_dram_parameter("tri", [128, 128], F32, isOutput=False)
    iorow = nc.declare_dram_parameter("iorow", [128, CAP], F32, isOutput=False)
    spart = nc.declare_dram_parameter("spart", [128, NSL + 1], F32, isOutput=False)
    p_in = nc.declare_dram_parameter("p_in", [TC, D], F32, isOutput=False)
    v_in = nc.declare_dram_parameter("v_in", [TC, D], F32, isOutput=False)
    m_in = nc.declare_dram_parameter("m_in", [TC, D], F32, isOutput=False)
    o_out = nc.declare_dram_parameter("o_out", [TC, D], F32, isOutput=True)
    o_p = nc.declare_dram_parameter("o_p", [TC, D], F32, isOutput=True)
    o_v = nc.declare_dram_parameter("o_v", [TC, D], F32, isOutput=True)
    o_m = nc.declare_dram_parameter("o_m", [TC, D], F32, isOutput=True)

    with tile.TileContext(nc) as tc:
        with (
            tc.tile_pool(name="wpool", bufs=1) as wpool,
            tc.tile_pool(name="wstream", bufs=2) as wstream,
            tc.tile_pool(name="xpool", bufs=1) as xpool,
            tc.tile_pool(name="gpool", bufs=1) as gpool,
            tc.tile_pool(name="cpool", bufs=1) as cpool,
            tc.tile_pool(name="fpool", bufs=1) as fpool,
            tc.tile_pool(name="epool", bufs=1) as epool,
            tc.tile_pool(name="psum", bufs=1, space="PSUM") as ppool,
            tc.tile_pool(name="dram", bufs=1, space="DRAM") as dpool,
        ):
            # ---- constants: split across scalar + gpsimd HWDGE rings ----
            gw_sb = wpool.tile([128, KD, E], F32)
            nc.scalar.dma_start(gw_sb, gw[:, :].rearrange("(k p) e -> p k e", p=128))
            gb_sb = wpool.tile([128, E], F32)
            nc.scalar.dma_start(gb_sb, _bcast_part(gbr[:, :], 128))
            tri_sb = wpool.tile([128, 128], F32)
            nc.gpsimd.dma_start(tri_sb, tri[:, :])
            iorow_sb = wpool.tile([128, CAP], F32)
            nc.gpsimd.dma_start(iorow_sb, iorow[:, :])
            spart_sb = wpool.tile([128, NSL + 1], F32)
            nc.gpsimd.dma_start(spart_sb, spart[:, :])
            b1_sb = wpool.tile([128, E * KH], F32)
            nc.scalar.dma_start(b1_sb, b1c[:, :])
            b2g = wpool.tile([E, D], BF16)
            nc.gpsimd.dma_start(b2g, b2r[:, :])
            lnwn_sb = wpool.tile([128, D], F32)
            nc.gpsimd.dma_start(lnwn_sb, _bcast_part(lnw[:, :], 128))
            nc.scalar.mul(lnwn_sb, lnwn_sb, -1.0)
            lnb_sb = wpool.tile([128, D], F32)
            nc.gpsimd.dma_start(lnb_sb, _bcast_part(lnb[:, :], 128))
            ones_row = wpool.tile([1, 128], F32)
            nc.vector.memset(ones_row, 1.0)
            ones_rbf = wpool.tile([1, 128], BF16)
            nc.vector.memset(ones_rbf, 1.0)
            ones_col = wpool.tile([128, 1], F32)
            nc.vector.memset(ones_col, 1.0)
            zeros_p1 = wpool.tile([128, 1], F32)
            nc.vector.memset(zeros_p1, 0.0)
            eps_adam_t = wpool.tile([128, 1], F32)
            nc.vector.memset(eps_adam_t, EPS_ADAM)
            eps_ln_t = wpool.tile([128, 1], F32)
            nc.vector.memset(eps_ln_t, EPS_LN)

            # ---- streamed inputs (sync HWDGE ring) ----
            # x^T arrives per token tile so the router can start on tile 0
            # while the rest is in flight
            xt_c = xpool.tile([128, KD, TC], F32)
            for tt in range(NTT):
                nc.sync.dma_start(
                    xt_c[:, :, tt * 128:(tt + 1) * 128],
                    xT[:, tt * 128:(tt + 1) * 128].rearrange(
                        "(k p) t -> p k t", p=128),
                )
            xbc = xpool.tile([128, NTT, D], BF16)
            nc.sync.dma_start(xbc, xb[:, :].rearrange("(tt p) d -> p tt d", p=128))

            # first experts' weights start streaming immediately; the host
            # pre-packs w1|w2 into the exact SBUF layout so each expert is a
            # single fully-contiguous DMA per matrix
            def load_weights(e):
                w1c = wstream.tile([128, KD * H], BF16, tag="w1c", bufs=2)
                nc.sync.dma_start(w1c, wcat[e, :, 0:KD * H])
                w2c = wstream.tile([128, KH * D], BF16, tag="w2c", bufs=2)
                nc.sync.dma_start(w2c, wcat[e, :, KD * H:])
                return (w1c, w2c)

            wts = [load_weights(e) for e in range(2)]

            # p/v/m prefetch: resident for the epilogue, loaded on the gpsimd
            # ring so they never contend with the weight stream or pile up in
            # the tail
            pvm_p = epool.tile([128, NTT, D], F32)
            nc.gpsimd.dma_start(pvm_p, p_in[:, :].rearrange("(tt p) d -> p tt d", p=128))
            pvm_v = epool.tile([128, NTT, D], F32)
            nc.gpsimd.dma_start(pvm_v, v_in[:, :].rearrange("(tt p) d -> p tt d", p=128))
            pvm_m = epool.tile([128, NTT, D], F32)
            nc.gpsimd.dma_start(pvm_m, m_in[:, :].rearrange("(tt p) d -> p tt d", p=128))

            # eo_all holds every expert's FFN output; zero the tail partitions
            # of the ragged second slot tile (scatter reads them against zero
            # selt rows — keep 0*garbage from becoming NaN)
            eo_all = fpool.tile([128, E, D], BF16)
            eo_pk = fpool.tile([128, E // 2, D], BF16, tag="eo_pk")
            nc.gpsimd.memset(eo_pk, 0.0)

            # ---- router: logits in fp32 [tokens, E] ----
            logit = gpool.tile([128, NTT, E], F32, tag="logit")
            for tt in range(NTT):
                ps_l = ppool.tile([128, 512], F32, tag="acc", bufs=4)
                for k in range(KD):
                    nc.tensor.matmul(
                        ps_l[:, 0:E],
                        xt_c[:, k, tt * 128:(tt + 1) * 128],
                        gw_sb[:, k, :],
                        start=(k == 0),
                        stop=(k == KD - 1),
                    )
                nc.vector.tensor_copy(logit[:, tt, :], ps_l[:, 0:E])
            gb3 = bass.AP(
                tensor=gb_sb.tensor, offset=gb_sb.offset,
                ap=[gb_sb.ap[0], [0, NTT], gb_sb.ap[1]],
            )
            nc.vector.tensor_tensor(logit, logit, gb3, ALU.add)

            # ---- top-2 softmax gates for all experts [tokens, E] ----
            m1 = gpool.tile([128, NTT, 1], F32, tag="m1")
            nc.vector.reduce_max(m1, logit, AX.X)
            m1b = _bcast_last(m1, E)
            lc = gpool.tile([128, NTT, E], F32, tag="lc")
            nc.vector.tensor_tensor(lc, logit, m1b, ALU.subtract)
            expl = gpool.tile([128, NTT, E], F32, tag="expl")
            nc.scalar.activation(expl, lc, ACTF.Exp, bias=zeros_p1, scale=1.0)
            mask1 = gpool.tile([128, NTT, E], F32, tag="mask1")
            nc.vector.tensor_tensor(mask1, logit, m1b, ALU.is_ge)
            l2 = gpool.tile([128, NTT, E], F32, tag="l2")
            nc.vector.scalar_tensor_tensor(
                l2, in0=mask1, scalar=-1e30, in1=logit, op0=ALU.mult, op1=ALU.add
            )
            m2 = gpool.tile([128, NTT, 1], F32, tag="m2")
            nc.vector.reduce_max(m2, l2, AX.X)
            mask2 = gpool.tile([128, NTT, E], F32, tag="mask2")
            nc.vector.tensor_tensor(mask2, logit, _bcast_last(m2, E), ALU.is_ge)
            ge = gpool.tile([128, NTT, E], F32, tag="ge")
            nc.vector.tensor_tensor(ge, expl, mask2, ALU.mult)
            den = gpool.tile([128, NTT, 1], F32, tag="den")
            nc.vector.reduce_sum(den, ge, AX.X)
            rden = gpool.tile([128, NTT, 1], F32, tag="rden")
            nc.vector.reciprocal(rden, den)
            gate = gpool.tile([128, NTT, E], F32, tag="gate")
            nc.vector.tensor_tensor(gate, ge, _bcast_last(rden, E), ALU.mult)
            gateb = gpool.tile([128, NTT, E], BF16, tag="gateb")
            nc.vector.tensor_copy(gateb, gate)

            # ---- compaction: per-expert slot ids via prefix-sum matmul ----
            mask = cpool.tile([128, NTT, E], F32, tag="mask")
            nc.vector.tensor_scalar(
                mask, in0=gate, scalar1=0.0, scalar2=None, op0=ALU.is_gt,
            )
            maskf = mask[:, :, :].rearrange("p a b -> p (a b)")
            ps_pos = ppool.tile([128, 512], F32, tag="acc", bufs=4)
            nc.tensor.matmul(ps_pos[:, 0:NTT * E], tri_sb[:, :], maskf,
                             start=True, stop=False)
            ps_cs = ppool.tile([128, 512], F32, tag="acc", bufs=4)
            nc.tensor.matmul(ps_cs[0:1, 0:NTT * E], ones_col[:, :], maskf,
                             start=True, stop=True)
            cs_sb = cpool.tile([1, NTT, E], F32, tag="cs_sb")
            nc.vector.tensor_copy(
                cs_sb, ps_cs[0:1, 0:NTT * E].rearrange("p (a b) -> p a b", a=NTT))
            excl = cpool.tile([1, NTT, E], F32, tag="excl")
            nc.vector.memset(excl[:, 0:1, :], 0.0)
            for tt in range(1, NTT):
                nc.vector.tensor_tensor(
                    excl[:, tt, :], excl[:, tt - 1, :], cs_sb[:, tt - 1, :], ALU.add,
                )
            nc.tensor.matmul(
                ps_pos[:, 0:NTT * E], ones_row[:, 0:128],
                excl[:, :, :].rearrange("p a b -> p (a b)"),
                start=False, stop=True,
            )
            # slotid = mask ? C_incl-1 : BIG
            sl_t1 = cpool.tile([128, NTT * E], F32, tag="sl_t1")
            nc.vector.tensor_scalar_add(sl_t1, ps_pos[:, 0:NTT * E], -1.0 - BIG)
            slotid = cpool.tile([128, NTT, E], F32, tag="slotid")
            slotf = slotid[:, :, :].rearrange("p a b -> p (a b)")
            nc.vector.tensor_tensor(slotf, sl_t1, maskf, ALU.mult)
            nc.vector.tensor_scalar_add(slotf, slotf, BIG)
            # bf16 copy for the DRAM bounce (ids ≤ 176 and 65536 are exact)
            slotidb = cpool.tile([128, NTT, E], BF16, tag="slotidb")
            nc.vector.tensor_copy(slotidb, slotid)

            # DRAM bounce: slotid & gate to the free axis, in bf16. DRAM
            # layout [(tt,e), p'] so per-expert row reads are contiguous runs.
            slotd = dpool.tile([NTT * E, 128], BF16, tag="slotd")
            slotd_t = bass.AP(tensor=slotd.tensor, offset=slotd.offset,
                              ap=[[1, 128], [128, NTT * E]])
            nc.sync.dma_start(slotd_t, slotidb[:, :, :].rearrange("p a b -> p (a b)"))
            gd = dpool.tile([NTT * E, 128], BF16, tag="gd")
            gd_t = bass.AP(tensor=gd.tensor, offset=gd.offset,
                           ap=[[1, 128], [128, NTT * E]])
            nc.sync.dma_start(gd_t, gateb[:, :, :].rearrange("p a b -> p (a b)"))

            # per-token-tile gate rows [E, 128] for the b2 starter matmul
            gt_sb = cpool.tile([E, NTT, 128], BF16, tag="gt_sb")
            nc.scalar.dma_start(
                gt_sb,
                bass.AP(tensor=gd.tensor, offset=gd.offset,
                        ap=[[128, E], [E * 128, NTT], [1, 128]]),
            )

            # Sel[token, slot] 0/1 bf16, slot axis = e*CAP + s (e-outer so the
            # gather's first 512-slot chunk unblocks after 3 experts)
            selm = cpool.tile([128, NTT, ECAP], BF16, tag="selm")
            for e in range(E):
                for tt in range(NTT):
                    nc.vector.tensor_tensor(
                        selm[:, tt, e * CAP:(e + 1) * CAP],
                        _bcast_last(slotid[:, tt, e:e + 1], CAP),
                        iorow_sb[:, :],
                        ALU.is_equal,
                    )

            selt = cpool.tile([128, E, TC], BF16, tag="selt")
            selt_pk = cpool.tile([128, E // 2, TC], BF16, tag="selt_pk")
            nc.gpsimd.memset(selt_pk, 0.0)

            def build_selt(e):
                # gate-weighted SelT'[slot, token]: read the expert's slot-id
                # and gate rows (tiny, bf16), broadcast across partitions with
                # a K=1 ones matmul into PSUM, then compare/scale on DVE.
                slotrb = cpool.tile([1, NTT, 128], BF16, tag="slotrb", bufs=2)
                nc.sync.dma_start(
                    slotrb,
                    bass.AP(tensor=slotd.tensor, offset=slotd.offset + e * 128,
                            ap=[[0, 1], [E * 128, NTT], [1, 128]]),
                )
                grb = cpool.tile([1, NTT, 128], BF16, tag="grb", bufs=2)
                nc.sync.dma_start(
                    grb,
                    bass.AP(tensor=gd.tensor, offset=gd.offset + e * 128,
                            ap=[[0, 1], [E * 128, NTT], [1, 128]]),
                )
                ps_sl = ppool.tile([128, 512], F32, tag="ps_g", bufs=1)
                nc.tensor.matmul(
                    ps_sl, ones_rbf[:, 0:128],
                    slotrb[:, :, :].rearrange("p a b -> p (a b)"),
                    start=True, stop=True,
                )
                ps_gt = ppool.tile([128, 512], F32, tag="ps_g", bufs=1)
                nc.tensor.matmul(
                    ps_gt, ones_rbf[:, 0:128],
                    grb[:, :, :].rearrange("p a b -> p (a b)"),
                    start=True, stop=True,
                )
                seltf = cpool.tile([128, TC], F32, tag="seltf", bufs=2)
                nc.vector.tensor_scalar(
                    seltf, in0=ps_sl, scalar1=spart_sb[:, 0:1],
                    scalar2=None, op0=ALU.is_equal,
                )
                nc.vector.tensor_tensor(
                    selt[:, e, :], seltf, ps_gt, ALU.mult
                )
                return slotrb, grb

            # PE warm-up: throwaway f32 matmuls fill the compaction lull so
            # the HAM clock gate is at K=8/8 when the gather stream begins
            ps_w = ppool.tile([128, 512], F32, tag="ps_g", bufs=1)
            for _ in range(14):
                nc.tensor.matmul(ps_w[:, 0:128], tri_sb[:, :], tri_sb[:, :],
                                 start=True, stop=True)

            # ---- gather-matmul: xgT[d, slot] = sum_t x[t,d]*Sel[t,slot] ----
            CH3 = [(0, 512), (512, 512), (1024, 384)]
            xgT = fpool.tile([128, KD, ECAP], BF16, tag="xgT")
            for n0, nn in CH3:
                for m in range(KD):
                    ps_g = ppool.tile([128, 512], F32, tag="acc", bufs=4)
                    for tt in range(NTT):
                        nc.tensor.matmul(
                            ps_g[:, :nn],
                            xbc[:, tt, m * 128:(m + 1) * 128],
                            selm[:, tt, n0:n0 + nn],
                            start=(tt == 0),
                            stop=(tt == NTT - 1),
                        )
                    nc.scalar.copy(xgT[:, m, n0:n0 + nn], ps_g[:, :nn])

            # ---- scatter accumulators: opened by the b2 starter matmul ----
            ps_sc = []
            for tt in range(NTT):
                t = ppool.tile([128, 512], F32, tag="acc", bufs=4)
                nc.tensor.matmul(
                    t, gt_sb[:, tt, :], b2g[:, :], start=True, stop=False,
                )
                ps_sc.append(t)

            pending = []   # deferred scatter matmuls: (selt_ap_fn, eo_ap, stop)

            def flush_pending():
                for sel_fn, eo_ap, stop in pending:
                    for tt in range(NTT):
                        nc.tensor.matmul(
                            ps_sc[tt],
                            sel_fn(tt),
                            eo_ap,
                            start=False,
                            stop=stop,
                        )
                pending.clear()

            # ---- per-expert FFN (weights stream through 2-buf tiles) ----
            pair_state = []
            for e in range(E):
                if e + 2 < E:
                    wts.append(load_weights(e + 2))
                w1c, w2c = wts[e]

                # matmul-1: hg = relu(xg @ w1 + b1), layout [H, slots]
                hg = fpool.tile([128, KH, CAP], BF16, tag="hg", bufs=2)
                for m in range(KH):
                    ps_h = ppool.tile([128, 512], F32, tag="ps_f", bufs=2)
                    for k in range(KD):
                        nc.tensor.matmul(
                            ps_h[:, :CAP],
                            w1c[:, k * H + m * 128:k * H + (m + 1) * 128],
                            xgT[:, k, e * CAP:(e + 1) * CAP],
                            start=(k == 0),
                            stop=(k == KD - 1),
                        )
                    nc.scalar.activation(
                        hg[:, m, :], ps_h[:, :CAP], ACTF.Relu,
                        bias=b1_sb[:, e * KH + m:e * KH + m + 1], scale=1.0,
                    )

                # previous expert/pair scatter now: inputs are long ready, and
                # it keeps the PE from stalling on this expert's DVE work
                flush_pending()

                # matmul-2 for the full first slot tile
                ps_o = ppool.tile([128, 512], F32, tag="ps_f", bufs=2)
                for k in range(KH):
                    nc.tensor.matmul(
                        ps_o,
                        hg[:, k, 0:128],
                        w2c[:, k * D:(k + 1) * D],
                        start=(k == 0),
                        stop=(k == KH - 1),
                    )
                nc.scalar.copy(eo_all[:, e, :], ps_o)

                rowtiles = build_selt(e)
                pair_state.append((hg, w2c, rowtiles))
                pending.append(
                    (lambda tt, e=e: selt[:, e, tt * 128:(tt + 1) * 128],
                     eo_all[:, e, :], False))

                if e % 2 == 1:
                    # ragged second slot tiles (48 slots) of the expert pair,
                    # column-packed at positions 0/64
                    pr = e // 2
                    (hgA, w2A, rtA), (hgB, w2B, rtB) = pair_state
                    pair_state.clear()
                    RS = CAP - 128
                    ps_pk = ppool.tile([128, 512], F32, tag="ps_pk", bufs=1)
                    for k in range(KH):
                        nc.tensor.matmul(
                            ps_pk[0:RS, :], hgA[:, k, 128:CAP],
                            w2A[:, k * D:(k + 1) * D],
                            start=(k == 0), stop=(k == KH - 1),
                            tile_position=(0, 0),
                        )
                        nc.tensor.matmul(
                            ps_pk[64:64 + RS, :], hgB[:, k, 128:CAP],
                            w2B[:, k * D:(k + 1) * D],
                            start=(k == 0), stop=(k == KH - 1),
                            tile_position=(0, 64),
                        )
                    nc.scalar.copy(eo_pk[0:RS, pr, :], ps_pk[0:RS, :])
                    nc.scalar.copy(eo_pk[64:64 + RS, pr, :], ps_pk[64:64 + RS, :])

                    # packed gate-weighted SelT rows for the pair
                    seltf = cpool.tile([128, TC], F32, tag="seltf", bufs=2)
                    for j, (srb, grb_) in enumerate((rtA, rtB)):
                        lo = 64 * j
                        ps_bs = ppool.tile([128, 512], F32, tag="ps_g", bufs=1)
                        nc.tensor.matmul(
                            ps_bs[lo:lo + RS, :], ones_rbf[:, 0:RS],
                            srb[:, :, :].rearrange("p a b -> p (a b)"),
                            start=True, stop=True, tile_position=(0, lo),
                        )
                        ps_bg = ppool.tile([128, 512], F32, tag="ps_g", bufs=1)
                        nc.tensor.matmul(
                            ps_bg[lo:lo + RS, :], ones_rbf[:, 0:RS],
                            grb_[:, :, :].rearrange("p a b -> p (a b)"),
                            start=True, stop=True, tile_position=(0, lo),
                        )
                        nc.vector.tensor_scalar(
                            seltf[lo:lo + RS, :], in0=ps_bs[lo:lo + RS, :],
                            scalar1=spart_sb[lo:lo + RS, 2:3],
                            scalar2=None, op0=ALU.is_equal,
                        )
                        nc.vector.tensor_tensor(
                            selt_pk[lo:lo + RS, pr, :], seltf[lo:lo + RS, :],
                            ps_bg[lo:lo + RS, :], ALU.mult,
                        )
                    pending.append(
                        (lambda tt, pr=pr: selt_pk[:, pr, tt * 128:(tt + 1) * 128],
                         eo_pk[:, pr, :], pr == E // 2 - 1))

            # ---- final scatter flush + fused Adam/LayerNorm epilogue ----
            # interleave per token tile so tt0's accumulation closes first and
            # its epilogue overlaps the remaining scatter matmuls
            assert len(pending) == 2
            for tt in range(NTT):
                for sel_fn, eo_ap, stop in pending:
                    nc.tensor.matmul(
                        ps_sc[tt], sel_fn(tt), eo_ap, start=False, stop=stop,
                    )
            pending.clear()

            out_rings = [nc.sync, nc.scalar, nc.gpsimd]
            for tt in range(NTT):
                rows = slice(tt * 128, (tt + 1) * 128)
                p_s = pvm_p[:, tt, :]
                v_s = pvm_v[:, tt, :]
                m_s = pvm_m[:, tt, :]
                eo_s = ps_sc[tt]

                t01 = epool.tile([128, D], F32, tag="tmp", bufs=2)
                nc.vector.tensor_scalar_mul(t01, eo_s, 1.0 - BETA1)
                pn = epool.tile([128, D], F32, tag="pn", bufs=2)
                nc.vector.scalar_tensor_tensor(
                    pn, in0=p_s, scalar=BETA1, in1=t01,
                    op0=ALU.mult, op1=ALU.add,
                )
                sq = epool.tile([128, D], F32, tag="tmp", bufs=2)
                nc.vector.scalar_tensor_tensor(
                    sq, in0=t01, scalar=(1.0 - BETA2) / (1.0 - BETA1) ** 2,
                    in1=t01, op0=ALU.mult, op1=ALU.mult,
                )
                vn = epool.tile([128, D], F32, tag="vn", bufs=2)
                nc.vector.scalar_tensor_tensor(
                    vn, in0=v_s, scalar=BETA2, in1=sq,
                    op0=ALU.mult, op1=ALU.add,
                )
                mo = epool.tile([128, D], F32, tag="mo", bufs=2)
                nc.vector.scalar_tensor_tensor(
                    mo, in0=m_s, scalar=MU, in1=eo_s,
                    op0=ALU.mult, op1=ALU.add,
                )

                r = epool.tile([128, D], F32, tag="tmp", bufs=2)
                nc.scalar.activation(r, vn, ACTF.Sqrt, bias=eps_adam_t, scale=1.0)
                nc.vector.reciprocal_approx_fast(r, r)
                yp = epool.tile([128, D], F32, tag="tmp", bufs=2)
                nc.vector.tensor_mul(yp, pn, r)
                stats = epool.tile([128, nc.vector.BN_STATS_DIM], F32, tag="st")
                nc.vector.bn_stats(stats, yp)
                mv = epool.tile([128, nc.vector.BN_AGGR_DIM], F32, tag="mv")
                nc.vector.bn_aggr(mv, stats)
                rstd = epool.tile([128, 1], F32, tag="rstd")
                nc.scalar.activation(
                    rstd, mv[:, 1:2], ACTF.Sqrt, bias=eps_ln_t, scale=1.0)
                nc.vector.reciprocal(rstd, rstd)
                nrm = epool.tile([128, D], F32, tag="tmp", bufs=2)
                nc.vector.tensor_scalar(
                    nrm, in0=yp, scalar1=mv[:, 0:1], scalar2=rstd,
                    op0=ALU.subtract, op1=ALU.mult,
                )
                o1 = epool.tile([128, D], F32, tag="tmp", bufs=2)
                nc.vector.tensor_mul(o1, nrm, lnwn_sb)
                oo = epool.tile([128, D], F32, tag="oo", bufs=2)
                nc.vector.tensor_add(oo, o1, lnb_sb)

                ring = out_rings[tt % len(out_rings)]
                ring.dma_start(o_out[rows, :], oo)
                ring.dma_start(o_p[rows, :], pn)
                ring.dma_start(o_v[rows, :], vn)
                ring.dma_start(o_m[rows, :], mo)

    nc.compile()
    return nc


_CACHED_NC = None


def _get_nc():
    global _CACHED_NC
    if _CACHED_NC is None:
        _CACHED_NC = build_graph()
    return _CACHED_NC


def run(inputs: dict, trace: bool = False):
    x = np.asarray(inputs["x"], np.float32).reshape(T, D)
    p = np.asarray(inputs["p"], np.float32).reshape(T, D)
    v = np.asarray(inputs["v"], np.float32).reshape(T, D)
    m = np.asarray(inputs["m"], np.float32).reshape(T, D)
    gate_w = np.asarray(inputs["gate_w"], np.float32)
    gate_b = np.asarray(inputs["gate_b"], np.float32)
    w1 = np.asarray(inputs["w1"], np.float32)
    b1 = np.asarray(inputs["b1"], np.float32)
    w2 = np.asarray(inputs["w2"], np.float32)
    b2 = np.asarray(inputs["b2"], np.float32)
    ln_w = np.asarray(inputs["ln_w"], np.float32)
    ln_b = np.asarray(inputs["ln_b"], np.float32)

    w1r = w1.reshape(E, KD, 128, H).transpose(0, 2, 1, 3).reshape(E, 128, KD * H)
    w2r = w2.reshape(E, KH, 128, D).transpose(0, 2, 1, 3).reshape(E, 128, KH * D)
    wcat = np.ascontiguousarray(
        np.concatenate([w1r, w2r], axis=2)).astype(ml_dtypes.bfloat16)
    # b1c[:, e*KH+m] = b1[e, m*128:(m+1)*128]
    b1c = np.ascontiguousarray(
        b1.reshape(E, KH, 128).transpose(2, 0, 1).reshape(128, E * KH))
    b2rm = np.ascontiguousarray(b2).astype(ml_dtypes.bfloat16)
    tri_m = np.triu(np.ones((128, 128), np.float32))
    iorow_m = np.broadcast_to(np.arange(CAP, dtype=np.float32), (128, CAP)).copy()
    pvals = np.arange(128, dtype=np.float32)
    spart_m = np.full((128, NSL + 1), -1.0, np.float32)
    for t2 in range(NSL):
        spart_m[:, t2] = t2 * 128 + pvals
    rs_ = CAP - 128
    for pp in range(128):
        if pp < rs_:
            spart_m[pp, NSL] = 128 + pp
        elif 64 <= pp < 64 + rs_:
            spart_m[pp, NSL] = 128 + (pp - 64)

    in_maps = []
    for i in range(NCORES):
        rows = slice(i * TC, (i + 1) * TC)
        in_maps.append({
            "xT": np.ascontiguousarray(x[rows].T),
            "xb": np.ascontiguousarray(x[rows]).astype(ml_dtypes.bfloat16),
            "wcat": wcat,
            "b1c": b1c,
            "b2r": b2rm,
            "gw": gate_w,
            "gbr": np.ascontiguousarray(gate_b[None, :]),
            "lnw": np.ascontiguousarray(ln_w[None, :]),
            "lnb": np.ascontiguousarray(ln_b[None, :]),
            "tri": tri_m,
            "iorow": iorow_m,
            "spart": spart_m,
            "p_in": np.ascontiguousarray(p[rows]),
            "v_in": np.ascontiguousarray(v[rows]),
            "m_in": np.ascontiguousarray(m[rows]),
        })

    nc = _get_nc()
    res = run_bass_kernel_spmd(nc, in_maps, core_ids=list(range(NCORES)), trace=trace)

    def gather(name: str) -> np.ndarray:
        full = np.empty((T, D), np.float32)
        for i in range(NCORES):
            full[i * TC:(i + 1) * TC] = res.results[i][name]
        return np.ascontiguousarray(full.reshape(B, S, D))

    outs = (gather("o_out"), gather("o_p"), gather("o_v"), gather("o_m"))
    return outs, res


def kernel(**inputs) -> tuple:
    outs, _ = run(inputs, trace=False)
    return outs
